# revision 11
# baseline (speedup 1.0000x reference)
"""Trainium2 Bass kernel for nn_CrossDomainMatching.

Computation (see reference):
  fea  = stack([x1.mean(L), x2.mean(L), x3.mean(L)])      # (B, 3, 256)
  x    = LayerNorm(fea)
  q/k/v projections (256x256), tiny 4-head attention over 3 queries x
  4 keys (3 projected + 1 register token), output projection, residual.
Outputs: (out + fea) (B, 3, 256) and match_score.mean(heads) (B, 3, 4).

Strategy: pure data parallel over batch across 8 cores (16 batches/core).
The only heavy part is streaming 3 x 512 MB from HBM for the mean; that is
done with fp32r ones-reduction matmuls on the tensor engine accumulating
into one PSUM bank (48 rows x 512 cols, even/odd L-interleaved), while the
tiny epilogue (LN + projections + attention) runs on vector/scalar/PE.

All small-weight preprocessing (transpose, LayerNorm gamma/beta folding,
q-scale) is done host-side in numpy.
"""

import math
import sys

import numpy as np

sys.path.insert(0, "/opt/trn_rl_repo")

import concourse.bacc as bacc  # noqa: E402
import concourse.bass as bass  # noqa: E402
import concourse.tile as tile  # noqa: E402
from concourse import mybir  # noqa: E402
from concourse.bass_utils import run_bass_kernel_spmd  # noqa: E402
from concourse.tile_sem_assignment import N_PROCS  # noqa: E402
from concourse.vector_clock import ScopedClock, VectorClock  # noqa: E402

B, L, DIM = 128, 4096, 256
HEADS, DIM_HEAD = 4, 64
INNER = HEADS * DIM_HEAD
SCALE = 1.0 / math.sqrt(float(DIM_HEAD))
LN_EPS = 1e-5

N_CORES = 8
BPC = B // N_CORES          # batches per core
R = BPC * 3                 # fea rows per core (48)
P = 128                     # sbuf partitions
CHUNK_F = L * DIM // P // 2  # 4096 f32 per partition: half a (b, t) block
N_MM = CHUNK_F // 512        # 8 ones-matmuls per chunk

F32 = mybir.dt.float32
F32R = mybir.dt.float32r
AX = mybir.AxisListType
OP = mybir.AluOpType
AF = mybir.ActivationFunctionType


class SplitDrainTileContext(tile.TileContext):
    """TileContext whose tail drain spreads its semaphore waits across
    one-wait-per-instruction sync nops: walrus rejects >2 sync waits on a
    single CTRL instruction, and the stock tail drain carries one wait per
    used semaphore lane."""

    def _drain_and_barrier(self, tick_clock, wait_clock):
        gc = tick_clock.global_clock
        for p in range(N_PROCS):
            if gc[p] <= 0:
                continue
            clk = VectorClock([gc[q] if q == p else 0 for q in range(N_PROCS)])
            nop = self.nc.sync.nop(nofuse=True, hint=f"split_drain_{p}")
            wait_clock.add_sem_waits(nop.ins, ScopedClock({None: clk}))
        self.nc.sync.drain()
        self.nc.all_engine_barrier()
        assert self.sems is not None
        popped = self.nc._tile_sem_poison_stack.pop()
        assert popped is self._sem_poison
        self.nc.clear_and_free_semaphores(list(self.sems.allocated().values()))
        self.nc.all_engine_barrier()


def _build_nc():
    nc = bacc.Bacc("TRN2", target_bir_lowering=False, debug=False,
                   num_devices=N_CORES)

    xs = [nc.dram_tensor(f"x{t + 1}", [BPC, L, DIM], F32R, kind="ExternalInput")
          for t in range(3)]
    wq = nc.dram_tensor("wq", [P, 512], F32, kind="ExternalInput")
    wk = nc.dram_tensor("wk", [P, 512], F32, kind="ExternalInput")
    wv = nc.dram_tensor("wv", [P, 512], F32, kind="ExternalInput")
    wo = nc.dram_tensor("wo", [P, 512], F32, kind="ExternalInput")
    # biases / register tokens arrive pre-broadcast to (R, 256)
    bcasts = {n: nc.dram_tensor(n, [R, 256], F32, kind="ExternalInput")
              for n in ("bq", "bk", "bv", "bo", "k3", "v3")}
    csel = nc.dram_tensor("csel", [P, R * 2 - 1], F32R, kind="ExternalInput")
    esel = nc.dram_tensor("esel", [R, 3 * R], F32, kind="ExternalInput")
    eye = nc.dram_tensor("eye", [P, P], F32, kind="ExternalInput")

    y1 = nc.dram_tensor("y1", [R, DIM], F32, kind="ExternalOutput")
    y2 = nc.dram_tensor("y2", [R, HEADS], F32, kind="ExternalOutput")

    with SplitDrainTileContext(nc) as tc:
        with (
            tc.tile_pool(name="consts", bufs=1) as consts,
            tc.tile_pool(name="stream", bufs=8) as stream,
            tc.tile_pool(name="ep", bufs=1) as ep,
            tc.tile_pool(name="pfea", bufs=1, space="PSUM") as pfea,
            tc.tile_pool(name="psmall", bufs=2, space="PSUM") as psmall,
        ):
            # ---- constants -------------------------------------------------
            # All constant DMAs go via SWDGE (gpsimd) so the HWDGE lane
            # rotation stays aligned with the streaming chunk slots.  Each
            # const tile gets a dummy same-engine consumer right after its
            # (single) DMA so real consumers inherit the dependency via
            # engine program order -- walrus rejects >1 sync wait on Matmult
            # and >2 on DMA instructions.
            csel_sb = consts.tile([P, R * 2 - 1], F32R)
            nc.gpsimd.dma_start(csel_sb[:], csel[:])
            pwarm = pfea.tile([R, 512], F32, tag="pwarm")
            nc.tensor.matmul(pwarm[:, 0:48], lhsT=csel_sb[:, 0:R],
                             rhs=csel_sb[:, 0:48], start=True, stop=True)
            esel_sb = consts.tile([R, 3 * R], F32)
            nc.gpsimd.dma_start(esel_sb[:], esel[:])
            nc.tensor.matmul(pwarm[:, 0:48], lhsT=esel_sb[:, 0:R],
                             rhs=esel_sb[:, 0:48], start=True, stop=True)
            eye_sb = consts.tile([P, P], F32)
            nc.gpsimd.dma_start(eye_sb[:], eye[:])
            nc.tensor.transpose(pwarm[:, 0:R], eye_sb[:R, :R], eye_sb[:R, :R])
            w_sb = {}
            for name, w in (("wq", wq), ("wk", wk), ("wv", wv), ("wo", wo)):
                t_ = consts.tile([P, 2 * 256], F32, tag=f"w_{name}")
                nc.gpsimd.dma_start(t_[:], w[:])
                nc.tensor.matmul(pwarm[:, 0:512], lhsT=t_[:, 0:48],
                                 rhs=t_[:, 0:512], start=True, stop=True)
                w_sb[name] = t_
            bc_sb = {}
            for n in ("bq", "bk", "bv", "bo", "k3", "v3"):
                t_ = consts.tile([R, 256], F32, tag=f"bc_{n}")
                nc.gpsimd.dma_start(t_[:], bcasts[n][:])
                scr = ep.tile([R, 1], F32, tag=f"bc_scr_{n}")
                nc.vector.tensor_copy(scr[:], t_[:, 0:1])
                bc_sb[n] = t_

            # ---- streaming mean reduction ---------------------------------
            # view each (L, DIM) block as (P, CHUNK_F): partition p holds 32
            # consecutive L-rows; matmul slice j covers L-rows p*32+2j+{0,1}
            # so psum col q*256+d accumulates sum over L-rows with parity q.
            psum_fea = pfea.tile([R, 512], F32)
            n_chunks = 3 * BPC * 2
            ci = 0
            for t in range(3):
                xr = xs[t].ap().rearrange("b (p s) d -> b p (s d)", p=P)
                for b in range(BPC):
                    r = b * 3 + t
                    lhs = csel_sb[:, R - 1 - r:2 * R - 1 - r]
                    for c in range(2):
                        ch = stream.tile([P, CHUNK_F], F32R, tag="ch")
                        nc.sync.dma_start(
                            ch[:], xr[b][:, c * CHUNK_F:(c + 1) * CHUNK_F])
                        for j in range(N_MM):
                            nc.tensor.matmul(
                                psum_fea[:, :],
                                lhsT=lhs,
                                rhs=ch[:, j * 512:(j + 1) * 512],
                                start=(ci == 0 and j == 0),
                                stop=(ci == n_chunks - 1 and j == N_MM - 1),
                            )
                        ci += 1

            # ---- epilogue --------------------------------------------------
            fea = ep.tile([R, 256], F32)
            feah = ep.tile([R, 256], F32)
            nc.vector.tensor_scalar_mul(feah[:], psum_fea[:, 256:512], 1.0 / L)
            nc.vector.scalar_tensor_tensor(
                out=fea[:], in0=psum_fea[:, 0:256], scalar=1.0 / L,
                in1=feah[:], op0=OP.mult, op1=OP.add)

            # LayerNorm statistics
            stats = ep.tile([R, 6], F32)
            nc.vector.bn_stats(out=stats[:], in_=fea[:])
            mv = ep.tile([R, 2], F32)
            nc.vector.bn_aggr(out=mv[:], in_=stats[:])
            eps_t = ep.tile([R, 1], F32)
            nc.vector.memset(eps_t[:], LN_EPS)
            rstd = ep.tile([R, 1], F32)
            nc.scalar.activation(rstd[:], mv[:, 1:2], AF.Sqrt, bias=eps_t[:, 0:1])
            nc.vector.reciprocal(rstd[:], rstd[:])
            normed = ep.tile([R, 256], F32)
            nc.vector.tensor_scalar(
                out=normed[:], in0=fea[:], scalar1=mv[:, 0:1],
                scalar2=rstd[:, 0:1], op0=OP.subtract, op1=OP.mult)

            # transpose normed -> xT (128, 2*R): d-major blocks
            xT = ep.tile([P, 2 * R], F32)
            for kblk in range(2):
                pt = psmall.tile([P, R], F32, tag="ptr")
                nc.tensor.transpose(
                    pt[:], normed[:, kblk * 128:(kblk + 1) * 128],
                    eye_sb[:R, :R])
                nc.scalar.copy(xT[:, kblk * R:(kblk + 1) * R], pt[:])

            # q/k/v projections (row-major outputs, bias folded via add)
            proj = {}
            for name, wn, bn in (("q", "wq", "bq"), ("k", "wk", "bk"),
                                 ("v", "wv", "bv")):
                pp = psmall.tile([R, 256], F32, tag="pproj")
                nc.tensor.matmul(pp[:], lhsT=xT[:, 0:R],
                                 rhs=w_sb[wn][:, 0:256], start=True, stop=False)
                nc.tensor.matmul(pp[:], lhsT=xT[:, R:2 * R],
                                 rhs=w_sb[wn][:, 256:512], start=False, stop=True)
                s = ep.tile([R, 256], F32, tag=f"proj_{name}")
                nc.vector.tensor_add(s[:], pp[:], bc_sb[bn][:])
                proj[name] = s

            # K_m / V_m: row-replicated key/value m-slices via selector matmuls
            kv_rows = {}
            for src, reg, pname in (("k", "k3", "K"), ("v", "v3", "V")):
                for m in range(3):
                    ps = psmall.tile([R, 256], F32, tag="psel")
                    nc.tensor.matmul(ps[:], lhsT=esel_sb[:, m * R:(m + 1) * R],
                                     rhs=proj[src][:], start=True, stop=True)
                    s = ep.tile([R, 256], F32, tag=f"{pname}{m}")
                    nc.vector.tensor_copy(s[:], ps[:])
                    kv_rows[(pname, m)] = s
                kv_rows[(pname, 3)] = bc_sb[reg]

            # gram: g[r, (h, m)] = sum_d q[r, (h, d)] * K_m[r, (h, d)]
            g = ep.tile([R, 16], F32)
            g3 = g[:].rearrange("p (h m) -> p h m", m=4)
            tmp = ep.tile([R, 256], F32)
            tmp3 = tmp[:].rearrange("p (h d) -> p h d", d=DIM_HEAD)
            for m in range(4):
                nc.vector.tensor_mul(tmp[:], proj["q"][:], kv_rows[("K", m)][:])
                nc.vector.tensor_reduce(out=g3[:, :, m:m + 1], in_=tmp3,
                                        axis=AX.X, op=OP.add)

            # softmax over m within each head
            mx = ep.tile([R, HEADS], F32)
            nc.vector.tensor_reduce(out=mx[:], in_=g3, axis=AX.X, op=OP.max)
            nc.vector.tensor_sub(g3, g3, mx[:, :, None].broadcast_to([R, 4, 4]))
            nc.scalar.activation(g[:], g[:], AF.Exp)
            sm = ep.tile([R, HEADS], F32)
            nc.vector.tensor_reduce(out=sm[:], in_=g3, axis=AX.X, op=OP.add)
            nc.vector.reciprocal(sm[:], sm[:])
            nc.vector.tensor_mul(g3, g3, sm[:, :, None].broadcast_to([R, 4, 4]))

            # attention output: attn[r, (h, d)] = sum_m score[r, (h, m)] * V_m
            attn = ep.tile([R, 256], F32)
            a3 = attn[:].rearrange("p (h d) -> p h d", d=DIM_HEAD)
            for m in range(4):
                v3v = kv_rows[("V", m)][:].rearrange("p (h d) -> p h d",
                                                     d=DIM_HEAD)
                scb = g3[:, :, m:m + 1].broadcast_to([R, 4, DIM_HEAD])
                if m == 0:
                    nc.vector.tensor_mul(a3, v3v, scb)
                else:
                    nc.vector.tensor_mul(tmp3, v3v, scb)
                    nc.vector.tensor_add(attn[:], attn[:], tmp[:])

            # transpose attn -> attnT, output projection, bias+residual
            attnT = ep.tile([P, 2 * R], F32)
            for kblk in range(2):
                pt = psmall.tile([P, R], F32, tag="ptr")
                nc.tensor.transpose(
                    pt[:], attn[:, kblk * 128:(kblk + 1) * 128], eye_sb[:R, :R])
                nc.scalar.copy(attnT[:, kblk * R:(kblk + 1) * R], pt[:])
            po = psmall.tile([R, 256], F32, tag="pproj")
            nc.tensor.matmul(po[:], lhsT=attnT[:, 0:R], rhs=w_sb["wo"][:, 0:256],
                             start=True, stop=False)
            nc.tensor.matmul(po[:], lhsT=attnT[:, R:2 * R],
                             rhs=w_sb["wo"][:, 256:512], start=False, stop=True)
            o = ep.tile([R, 256], F32)
            nc.vector.tensor_add(o[:], po[:], bc_sb["bo"][:])
            nc.vector.tensor_add(o[:], o[:], fea[:])
            nc.sync.dma_start(y1[:], o[:])

            # match_score.mean(heads): reduce h (stride-4 cols) per m
            msm = ep.tile([R, 4], F32)
            g_mh = g[:].rearrange("p (h m) -> p m h", m=4)
            nc.vector.tensor_reduce(out=msm[:], in_=g_mh, axis=AX.X, op=OP.add)
            nc.scalar.mul(msm[:], msm[:], 1.0 / HEADS)
            nc.sync.dma_start(y2[:], msm[:])

    nc.compile()
    return nc


_NC_CACHE = {}


def _get_nc():
    if "nc" not in _NC_CACHE:
        _NC_CACHE["nc"] = _build_nc()
    return _NC_CACHE["nc"]


def _host_prep(inputs):
    f = lambda k: np.asarray(inputs[k], np.float32)
    ln_w, ln_b = f("ln_w"), f("ln_b")
    W_theta, W_phi, W_mu, W_out = f("W_theta"), f("W_phi"), f("W_mu"), f("W_out")
    b_theta, b_phi, b_mu, b_out = f("b_theta"), f("b_phi"), f("b_mu"), f("b_out")
    reg_phi = f("reg_phi").reshape(INNER)
    reg_mu = f("reg_mu").reshape(INNER)

    def wlayout(wt):
        # (256 d, 256 i) -> (128 p, 2*256): cols [kblk*256 + i], d = kblk*128+p
        return np.ascontiguousarray(
            wt.reshape(2, P, 256).transpose(1, 0, 2).reshape(P, 512))

    wq_t = wlayout((W_theta * ln_w[None, :]).T * SCALE)
    wk_t = wlayout((W_phi * ln_w[None, :]).T)
    wv_t = wlayout((W_mu * ln_w[None, :]).T)
    wo_t = wlayout(W_out.T)
    bq = (b_theta + ln_b @ W_theta.T) * SCALE
    bk = b_phi + ln_b @ W_phi.T
    bv = b_mu + ln_b @ W_mu.T

    bc = lambda v: np.ascontiguousarray(
        np.broadcast_to(v[None, :], (R, 256)).astype(np.float32))

    csel = np.zeros((P, R * 2 - 1), np.float32)
    csel[:, R - 1] = 1.0
    esel = np.zeros((R, 3 * R), np.float32)
    for m in range(3):
        for r in range(R):
            esel[3 * (r // 3) + m, m * R + r] = 1.0
    eye = np.eye(P, dtype=np.float32)

    common = {
        "wq": wq_t, "wk": wk_t, "wv": wv_t, "wo": wo_t,
        "bq": bc(bq), "bk": bc(bk), "bv": bc(bv), "bo": bc(b_out),
        "k3": bc(reg_phi), "v3": bc(reg_mu),
        "csel": csel, "esel": esel, "eye": eye,
    }
    return common


def kernel(**inputs):
    x1 = np.asarray(inputs["x1"], np.float32)
    x2 = np.asarray(inputs["x2"], np.float32)
    x3 = np.asarray(inputs["x3"], np.float32)
    common = _host_prep(inputs)

    in_maps = []
    for c in range(N_CORES):
        sl = slice(c * BPC, (c + 1) * BPC)
        m = dict(common)
        m["x1"] = np.ascontiguousarray(x1[sl])
        m["x2"] = np.ascontiguousarray(x2[sl])
        m["x3"] = np.ascontiguousarray(x3[sl])
        in_maps.append(m)

    nc = _get_nc()
    res = run_bass_kernel_spmd(nc, in_maps, list(range(N_CORES)))
    y1 = np.concatenate(
        [res.results[c]["y1"].reshape(BPC, 3, DIM) for c in range(N_CORES)], axis=0)
    y2 = np.concatenate(
        [res.results[c]["y2"].reshape(BPC, 3, HEADS) for c in range(N_CORES)], axis=0)
    return y1, y2


# revision 13
# speedup vs baseline: 2.3644x; 2.3644x over previous
"""Trainium2 Bass kernel for nn_CrossDomainMatching.

Computation (see reference):
  fea  = stack([x1.mean(L), x2.mean(L), x3.mean(L)])      # (B, 3, 256)
  x    = LayerNorm(fea)
  q/k/v projections (256x256), tiny 4-head attention over 3 queries x
  4 keys (3 projected + 1 register token), output projection, residual.
Outputs: (out + fea) (B, 3, 256) and match_score.mean(heads) (B, 3, 4).

Strategy: pure data parallel over batch across 8 cores (16 batches/core).
The only heavy part is streaming 3 x 512 MB from HBM for the mean; that is
done with fp32r ones-reduction matmuls on the tensor engine accumulating
into one PSUM bank (48 rows x 512 cols, even/odd L-interleaved), while the
tiny epilogue (LN + projections + attention) runs on vector/scalar/PE.

All small-weight preprocessing (transpose, LayerNorm gamma/beta folding,
q-scale) is done host-side in numpy.
"""

import math
import sys

import numpy as np

sys.path.insert(0, "/opt/trn_rl_repo")

import concourse.bacc as bacc  # noqa: E402
import concourse.bass as bass  # noqa: E402
import concourse.tile as tile  # noqa: E402
from concourse import mybir  # noqa: E402
from concourse.bass_utils import run_bass_kernel_spmd  # noqa: E402
from concourse.tile_sem_assignment import N_PROCS  # noqa: E402
from concourse.vector_clock import ScopedClock, VectorClock  # noqa: E402

B, L, DIM = 128, 4096, 256
HEADS, DIM_HEAD = 4, 64
INNER = HEADS * DIM_HEAD
SCALE = 1.0 / math.sqrt(float(DIM_HEAD))
LN_EPS = 1e-5

N_CORES = 8
BPC = B // N_CORES          # batches per core
R = BPC * 3                 # fea rows per core (48)
P = 128                     # sbuf partitions
CHUNK_F = L * DIM // P // 2  # 4096 f32 per partition: half a (b, t) block
N_MM = CHUNK_F // 512        # 8 ones-matmuls per chunk

F32 = mybir.dt.float32
F32R = mybir.dt.float32r
AX = mybir.AxisListType
OP = mybir.AluOpType
AF = mybir.ActivationFunctionType


class SplitDrainTileContext(tile.TileContext):
    """TileContext whose tail drain spreads its semaphore waits across
    one-wait-per-instruction sync nops: walrus rejects >2 sync waits on a
    single CTRL instruction, and the stock tail drain carries one wait per
    used semaphore lane."""

    def _drain_and_barrier(self, tick_clock, wait_clock):
        gc = tick_clock.global_clock
        for p in range(N_PROCS):
            if gc[p] <= 0:
                continue
            clk = VectorClock([gc[q] if q == p else 0 for q in range(N_PROCS)])
            nop = self.nc.sync.nop(nofuse=True, hint=f"split_drain_{p}")
            wait_clock.add_sem_waits(nop.ins, ScopedClock({None: clk}))
        self.nc.sync.drain()
        self.nc.all_engine_barrier()
        assert self.sems is not None
        popped = self.nc._tile_sem_poison_stack.pop()
        assert popped is self._sem_poison
        self.nc.clear_and_free_semaphores(list(self.sems.allocated().values()))
        self.nc.all_engine_barrier()


def _build_nc(chunks_per_block=2):
    nc = bacc.Bacc("TRN2", target_bir_lowering=False, debug=False,
                   num_devices=N_CORES)

    xs = [nc.dram_tensor(f"x{t + 1}", [BPC, L, DIM], F32R, kind="ExternalInput")
          for t in range(3)]
    wq = nc.dram_tensor("wq", [P, 512], F32, kind="ExternalInput")
    wk = nc.dram_tensor("wk", [P, 512], F32, kind="ExternalInput")
    wv = nc.dram_tensor("wv", [P, 512], F32, kind="ExternalInput")
    wo = nc.dram_tensor("wo", [P, 512], F32, kind="ExternalInput")
    # biases / register tokens arrive pre-broadcast to (R, 256)
    bcasts = {n: nc.dram_tensor(n, [R, 256], F32, kind="ExternalInput")
              for n in ("bq", "bk", "bv", "bo", "k3", "v3")}
    csel = nc.dram_tensor("csel", [P, R * 2 - 1], F32R, kind="ExternalInput")
    esel = nc.dram_tensor("esel", [R, 3 * R], F32, kind="ExternalInput")
    eye = nc.dram_tensor("eye", [P, P], F32, kind="ExternalInput")

    y1 = nc.dram_tensor("y1", [R, DIM], F32, kind="ExternalOutput")
    y2 = nc.dram_tensor("y2", [R, HEADS], F32, kind="ExternalOutput")

    with SplitDrainTileContext(nc) as tc:
        with (
            tc.tile_pool(name="consts", bufs=1) as consts,
            tc.tile_pool(name="stream", bufs=8) as stream,
            tc.tile_pool(name="ep", bufs=1) as ep,
            tc.tile_pool(name="pfea", bufs=1, space="PSUM") as pfea,
            tc.tile_pool(name="psmall", bufs=2, space="PSUM") as psmall,
        ):
            # ---- constants -------------------------------------------------
            # All constant DMAs go via SWDGE (gpsimd) so the HWDGE lane
            # rotation stays aligned with the streaming chunk slots.  Each
            # const tile gets a dummy same-engine consumer right after its
            # (single) DMA so real consumers inherit the dependency via
            # engine program order -- walrus rejects >1 sync wait on Matmult
            # and >2 on DMA instructions.
            csel_sb = consts.tile([P, R * 2 - 1], F32R)
            nc.gpsimd.dma_start(csel_sb[:], csel[:])
            pwarm = pfea.tile([R, 512], F32, tag="pwarm")
            nc.tensor.matmul(pwarm[:, 0:48], lhsT=csel_sb[:, 0:R],
                             rhs=csel_sb[:, 0:48], start=True, stop=True)
            esel_sb = consts.tile([R, 3 * R], F32)
            nc.gpsimd.dma_start(esel_sb[:], esel[:])
            nc.tensor.matmul(pwarm[:, 0:48], lhsT=esel_sb[:, 0:R],
                             rhs=esel_sb[:, 0:48], start=True, stop=True)
            eye_sb = consts.tile([P, P], F32)
            nc.gpsimd.dma_start(eye_sb[:], eye[:])
            nc.tensor.transpose(pwarm[:, 0:R], eye_sb[:R, :R], eye_sb[:R, :R])
            w_sb = {}
            for name, w in (("wq", wq), ("wk", wk), ("wv", wv), ("wo", wo)):
                t_ = consts.tile([P, 2 * 256], F32, tag=f"w_{name}")
                nc.gpsimd.dma_start(t_[:], w[:])
                nc.tensor.matmul(pwarm[:, 0:512], lhsT=t_[:, 0:48],
                                 rhs=t_[:, 0:512], start=True, stop=True)
                w_sb[name] = t_
            bc_sb = {}
            for n in ("bq", "bk", "bv", "bo", "k3", "v3"):
                t_ = consts.tile([R, 256], F32, tag=f"bc_{n}")
                nc.gpsimd.dma_start(t_[:], bcasts[n][:])
                scr = ep.tile([R, 1], F32, tag=f"bc_scr_{n}")
                nc.vector.tensor_copy(scr[:], t_[:, 0:1])
                bc_sb[n] = t_

            # ---- streaming mean reduction ---------------------------------
            # view each (L, DIM) block as (P, CHUNK_F): partition p holds 32
            # consecutive L-rows; matmul slice j covers L-rows p*32+2j+{0,1}
            # so psum col q*256+d accumulates sum over L-rows with parity q.
            psum_fea = pfea.tile([R, 512], F32)
            n_chunks = 3 * BPC * chunks_per_block
            ci = 0
            for t in range(3):
                xr = xs[t].ap().rearrange("b (p s) d -> b p (s d)", p=P)
                for b in range(BPC):
                    r = b * 3 + t
                    lhs = csel_sb[:, R - 1 - r:2 * R - 1 - r]
                    for c in range(chunks_per_block):
                        ch = stream.tile([P, CHUNK_F], F32R, tag="ch")
                        nc.sync.dma_start(
                            ch[:], xr[b][:, c * CHUNK_F:(c + 1) * CHUNK_F])
                        for j in range(N_MM):
                            nc.tensor.matmul(
                                psum_fea[:, :],
                                lhsT=lhs,
                                rhs=ch[:, j * 512:(j + 1) * 512],
                                start=(ci == 0 and j == 0),
                                stop=(ci == n_chunks - 1 and j == N_MM - 1),
                            )
                        ci += 1

            # ---- epilogue --------------------------------------------------
            if chunks_per_block == 0:
                nc.tensor.matmul(psum_fea[:, 0:48], lhsT=csel_sb[:, 0:R],
                                 rhs=csel_sb[:, 0:48], start=True, stop=True)
            fea = ep.tile([R, 256], F32)
            feah = ep.tile([R, 256], F32)
            nc.vector.tensor_scalar_mul(feah[:], psum_fea[:, 256:512], 1.0 / L)
            nc.vector.scalar_tensor_tensor(
                out=fea[:], in0=psum_fea[:, 0:256], scalar=1.0 / L,
                in1=feah[:], op0=OP.mult, op1=OP.add)

            # LayerNorm statistics
            stats = ep.tile([R, 6], F32)
            nc.vector.bn_stats(out=stats[:], in_=fea[:])
            mv = ep.tile([R, 2], F32)
            nc.vector.bn_aggr(out=mv[:], in_=stats[:])
            eps_t = ep.tile([R, 1], F32)
            nc.vector.memset(eps_t[:], LN_EPS)
            rstd = ep.tile([R, 1], F32)
            nc.scalar.activation(rstd[:], mv[:, 1:2], AF.Sqrt, bias=eps_t[:, 0:1])
            nc.vector.reciprocal(rstd[:], rstd[:])
            normed = ep.tile([R, 256], F32)
            nc.vector.tensor_scalar(
                out=normed[:], in0=fea[:], scalar1=mv[:, 0:1],
                scalar2=rstd[:, 0:1], op0=OP.subtract, op1=OP.mult)

            # transpose normed -> xT (128, 2*R): d-major blocks
            xT = ep.tile([P, 2 * R], F32)
            for kblk in range(2):
                pt = psmall.tile([P, R], F32, tag="ptr")
                nc.tensor.transpose(
                    pt[:], normed[:, kblk * 128:(kblk + 1) * 128],
                    eye_sb[:R, :R])
                nc.scalar.copy(xT[:, kblk * R:(kblk + 1) * R], pt[:])

            # q/k/v projections (row-major outputs, bias folded via add)
            proj = {}
            for name, wn, bn in (("q", "wq", "bq"), ("k", "wk", "bk"),
                                 ("v", "wv", "bv")):
                pp = psmall.tile([R, 256], F32, tag="pproj")
                nc.tensor.matmul(pp[:], lhsT=xT[:, 0:R],
                                 rhs=w_sb[wn][:, 0:256], start=True, stop=False)
                nc.tensor.matmul(pp[:], lhsT=xT[:, R:2 * R],
                                 rhs=w_sb[wn][:, 256:512], start=False, stop=True)
                s = ep.tile([R, 256], F32, tag=f"proj_{name}")
                nc.vector.tensor_add(s[:], pp[:], bc_sb[bn][:])
                proj[name] = s

            # K_m / V_m: row-replicated key/value m-slices via selector matmuls
            kv_rows = {}
            for src, reg, pname in (("k", "k3", "K"), ("v", "v3", "V")):
                for m in range(3):
                    ps = psmall.tile([R, 256], F32, tag="psel")
                    nc.tensor.matmul(ps[:], lhsT=esel_sb[:, m * R:(m + 1) * R],
                                     rhs=proj[src][:], start=True, stop=True)
                    s = ep.tile([R, 256], F32, tag=f"{pname}{m}")
                    nc.vector.tensor_copy(s[:], ps[:])
                    kv_rows[(pname, m)] = s
                kv_rows[(pname, 3)] = bc_sb[reg]

            # gram: g[r, (h, m)] = sum_d q[r, (h, d)] * K_m[r, (h, d)]
            g = ep.tile([R, 16], F32)
            g3 = g[:].rearrange("p (h m) -> p h m", m=4)
            tmp = ep.tile([R, 256], F32)
            tmp3 = tmp[:].rearrange("p (h d) -> p h d", d=DIM_HEAD)
            for m in range(4):
                nc.vector.tensor_mul(tmp[:], proj["q"][:], kv_rows[("K", m)][:])
                nc.vector.tensor_reduce(out=g3[:, :, m:m + 1], in_=tmp3,
                                        axis=AX.X, op=OP.add)

            # softmax over m within each head
            mx = ep.tile([R, HEADS], F32)
            nc.vector.tensor_reduce(out=mx[:], in_=g3, axis=AX.X, op=OP.max)
            nc.vector.tensor_sub(g3, g3, mx[:, :, None].broadcast_to([R, 4, 4]))
            nc.scalar.activation(g[:], g[:], AF.Exp)
            sm = ep.tile([R, HEADS], F32)
            nc.vector.tensor_reduce(out=sm[:], in_=g3, axis=AX.X, op=OP.add)
            nc.vector.reciprocal(sm[:], sm[:])
            nc.vector.tensor_mul(g3, g3, sm[:, :, None].broadcast_to([R, 4, 4]))

            # attention output: attn[r, (h, d)] = sum_m score[r, (h, m)] * V_m
            attn = ep.tile([R, 256], F32)
            a3 = attn[:].rearrange("p (h d) -> p h d", d=DIM_HEAD)
            for m in range(4):
                v3v = kv_rows[("V", m)][:].rearrange("p (h d) -> p h d",
                                                     d=DIM_HEAD)
                scb = g3[:, :, m:m + 1].broadcast_to([R, 4, DIM_HEAD])
                if m == 0:
                    nc.vector.tensor_mul(a3, v3v, scb)
                else:
                    nc.vector.tensor_mul(tmp3, v3v, scb)
                    nc.vector.tensor_add(attn[:], attn[:], tmp[:])

            # transpose attn -> attnT, output projection, bias+residual
            attnT = ep.tile([P, 2 * R], F32)
            for kblk in range(2):
                pt = psmall.tile([P, R], F32, tag="ptr")
                nc.tensor.transpose(
                    pt[:], attn[:, kblk * 128:(kblk + 1) * 128], eye_sb[:R, :R])
                nc.scalar.copy(attnT[:, kblk * R:(kblk + 1) * R], pt[:])
            po = psmall.tile([R, 256], F32, tag="pproj")
            nc.tensor.matmul(po[:], lhsT=attnT[:, 0:R], rhs=w_sb["wo"][:, 0:256],
                             start=True, stop=False)
            nc.tensor.matmul(po[:], lhsT=attnT[:, R:2 * R],
                             rhs=w_sb["wo"][:, 256:512], start=False, stop=True)
            o = ep.tile([R, 256], F32)
            nc.vector.tensor_add(o[:], po[:], bc_sb["bo"][:])
            nc.vector.tensor_add(o[:], o[:], fea[:])
            nc.sync.dma_start(y1[:], o[:])

            # match_score.mean(heads): reduce h (stride-4 cols) per m
            msm = ep.tile([R, 4], F32)
            g_mh = g[:].rearrange("p (h m) -> p m h", m=4)
            nc.vector.tensor_reduce(out=msm[:], in_=g_mh, axis=AX.X, op=OP.add)
            nc.scalar.mul(msm[:], msm[:], 1.0 / HEADS)
            nc.sync.dma_start(y2[:], msm[:])

    nc.compile()
    return nc


_NC_CACHE = {}


def _get_nc():
    if "nc" not in _NC_CACHE:
        _NC_CACHE["nc"] = _build_nc()
    return _NC_CACHE["nc"]


def _host_prep(inputs):
    f = lambda k: np.asarray(inputs[k], np.float32)
    ln_w, ln_b = f("ln_w"), f("ln_b")
    W_theta, W_phi, W_mu, W_out = f("W_theta"), f("W_phi"), f("W_mu"), f("W_out")
    b_theta, b_phi, b_mu, b_out = f("b_theta"), f("b_phi"), f("b_mu"), f("b_out")
    reg_phi = f("reg_phi").reshape(INNER)
    reg_mu = f("reg_mu").reshape(INNER)

    def wlayout(wt):
        # (256 d, 256 i) -> (128 p, 2*256): cols [kblk*256 + i], d = kblk*128+p
        return np.ascontiguousarray(
            wt.reshape(2, P, 256).transpose(1, 0, 2).reshape(P, 512))

    wq_t = wlayout((W_theta * ln_w[None, :]).T * SCALE)
    wk_t = wlayout((W_phi * ln_w[None, :]).T)
    wv_t = wlayout((W_mu * ln_w[None, :]).T)
    wo_t = wlayout(W_out.T)
    bq = (b_theta + ln_b @ W_theta.T) * SCALE
    bk = b_phi + ln_b @ W_phi.T
    bv = b_mu + ln_b @ W_mu.T

    bc = lambda v: np.ascontiguousarray(
        np.broadcast_to(v[None, :], (R, 256)).astype(np.float32))

    csel = np.zeros((P, R * 2 - 1), np.float32)
    csel[:, R - 1] = 1.0
    esel = np.zeros((R, 3 * R), np.float32)
    for m in range(3):
        for r in range(R):
            esel[3 * (r // 3) + m, m * R + r] = 1.0
    eye = np.eye(P, dtype=np.float32)

    common = {
        "wq": wq_t, "wk": wk_t, "wv": wv_t, "wo": wo_t,
        "bq": bc(bq), "bk": bc(bk), "bv": bc(bv), "bo": bc(b_out),
        "k3": bc(reg_phi), "v3": bc(reg_mu),
        "csel": csel, "esel": esel, "eye": eye,
    }
    return common


def kernel(**inputs):
    x1 = np.asarray(inputs["x1"], np.float32)
    x2 = np.asarray(inputs["x2"], np.float32)
    x3 = np.asarray(inputs["x3"], np.float32)
    common = _host_prep(inputs)

    in_maps = []
    for c in range(N_CORES):
        sl = slice(c * BPC, (c + 1) * BPC)
        m = dict(common)
        m["x1"] = np.ascontiguousarray(x1[sl])
        m["x2"] = np.ascontiguousarray(x2[sl])
        m["x3"] = np.ascontiguousarray(x3[sl])
        in_maps.append(m)

    nc = _get_nc()
    res = run_bass_kernel_spmd(nc, in_maps, list(range(N_CORES)))
    y1 = np.concatenate(
        [res.results[c]["y1"].reshape(BPC, 3, DIM) for c in range(N_CORES)], axis=0)
    y2 = np.concatenate(
        [res.results[c]["y2"].reshape(BPC, 3, HEADS) for c in range(N_CORES)], axis=0)
    return y1, y2
